# revision 5
# baseline (speedup 1.0000x reference)
"""Trainium2 Bass kernel for a 3-layer GAT (PyG GATConv-style) over 8 NeuronCores.

Strategy (graph/data parallel, per the sharding hint):
  - Nodes are partitioned across the 8 cores (6250 owned nodes per core).
  - Per layer: each core computes h = x @ W for its own nodes (dense phase),
    plus per-head attention logit pieces exp(a_s.h) / exp(0.2 a_s.h) (source
    side, packed next to h in a "gather table" row) and exp(a_d.h) /
    exp(0.2 a_d.h) (dest side, in a small local table).
  - The per-core table shards are AllGathered so each core holds the full
    [50176, row] table in its HBM.
  - Each core then aggregates its own destination nodes: per-edge rows are
    fetched with indirect DMA (dma_gather), per-edge softmax weights are
    q = max(u1*v1, u2*v2)  (= exp(leaky_relu(a_s.h_src + a_d.h_dst)), using
    exp(leaky(z)) = max(exp(z), exp(0.2 z)) since leaky(z) = max(z, 0.2z)),
    and the weighted segment-sum over incoming edges is computed on the
    tensor engine as  psum[dst, :] += ind(e, dst)^T @ (q * h_gathered)
    where ind is a 0/1 edge->dst-slot matrix built with one DVE is_equal op.
    The softmax denominator comes from the same matmul with rhs = q.
    Softmax max-subtraction is skipped: attention logits for this problem
    are O(10), far inside fp32 exp range, so the result is identical.
  - Edges are host-side packed into 49 groups of 128 destination slots per
    core with a uniform chunk budget so all 8 cores run one SPMD program.
"""

import numpy as np

# ---------------------------------------------------------------- config

F32MM = "float32"  # tensor-engine dtype for matmuls: "float32" (4x slower) or "float32r"


def make_cfg(n_nodes=50000, n_cores=8, ca=5, groups=None):
    cfg = {}
    cfg["N"] = n_nodes
    cfg["NC"] = n_cores
    cfg["NOWN"] = n_nodes // n_cores
    g = (cfg["NOWN"] + 127) // 128 if groups is None else groups
    cfg["G"] = g
    cfg["NPAD"] = g * 128
    cfg["NTAB"] = cfg["NPAD"] * n_cores
    assert cfg["NTAB"] % 2 == 0
    cfg["HALF"] = cfg["NTAB"] // 2
    assert cfg["HALF"] <= 32767 + 1, "int16 gather index limit"
    cfg["CA"] = ca  # chunks per (group, table-half) piece
    cfg["KC"] = 2 * ca
    # layers: (F_in, heads, head_dim, F_out, table_width, elu)
    cfg["layers"] = [
        (256, 8, 128, 1024, 1088, True),
        (1024, 4, 128, 512, 576, True),
        (512, 1, 64, 64, 128, False),
    ]
    return cfg


# ------------------------------------------------------- host preprocessing


def _pack_bins(degA, degB, n_groups, cap):
    """Greedy first-fit-decreasing: assign nodes to groups of <=128 nodes with
    per-group edge budgets sum(degA) <= cap and sum(degB) <= cap."""
    n = len(degA)
    order = np.argsort(-(degA + degB), kind="stable")
    loadA = np.zeros(n_groups)
    loadB = np.zeros(n_groups)
    cnt = np.zeros(n_groups, dtype=np.int64)
    assign = np.full(n, -1, dtype=np.int64)
    for nd in order:
        la = loadA + degA[nd]
        lb = loadB + degB[nd]
        ok = (cnt < 128) & (la <= cap) & (lb <= cap)
        if not ok.any():
            return None
        score = np.maximum(la / cap, lb / cap) + cnt * 1e-4
        score[~ok] = np.inf
        b = int(np.argmin(score))
        assign[nd] = b
        loadA[b] = la[b]
        loadB[b] = lb[b]
        cnt[b] += 1
    return assign


def _wrap_idx(arr):
    """dma_gather index layout: idx i -> partition i%16, col i//16; replicated
    to 128 partitions (8 gpsimd cores x 16)."""
    w = arr.reshape(-1, 16).T.astype(np.int16)  # [16, n/16]
    return np.tile(w, (8, 1))  # [128, n/16]


def preprocess(x, edge_index, cfg):
    N, NC, NOWN, G = cfg["N"], cfg["NC"], cfg["NOWN"], cfg["G"]
    NPAD, HALF, CA, KC = cfg["NPAD"], cfg["HALF"], cfg["CA"], cfg["KC"]

    src = np.asarray(edge_index[0], dtype=np.int64)
    dst = np.asarray(edge_index[1], dtype=np.int64)
    loops = np.arange(N, dtype=np.int64)
    src = np.concatenate([src, loops])
    dst = np.concatenate([dst, loops])

    src_core = src // NOWN
    dst_core = dst // NOWN

    per_core = []
    ca = CA
    for k in range(NC):
        mask = dst_core == k
        es, ed = src[mask], dst[mask]
        local_dst = ed - k * NOWN
        # split degree by which table half the source row lands in
        # (src core < NC/2 -> half A); independent of local permutations.
        a_edge = src_core[mask] < (NC // 2)
        degA = np.bincount(local_dst[a_edge], minlength=NOWN)
        degB = np.bincount(local_dst[~a_edge], minlength=NOWN)
        assign = _pack_bins(degA, degB, G, ca * 128)
        while assign is None:
            ca += 1
            assign = _pack_bins(degA, degB, G, ca * 128)
        per_core.append((es, ed, local_dst, a_edge, assign))
    if ca != CA:
        cfg["CA"] = ca
        cfg["KC"] = 2 * ca
        CA, KC = ca, 2 * ca

    pre = []
    # global -> (core, local position) map; needs all cores' assignments
    local_pos_of = np.empty(N, dtype=np.int64)
    orders = []
    for k in range(NC):
        _, _, _, _, assign = per_core[k]
        order = np.full(NPAD, -1, dtype=np.int64)  # local pos -> global id
        slot_in_bin = np.zeros(G, dtype=np.int64)
        for nd in range(NOWN):
            b = assign[nd]
            p = b * 128 + slot_in_bin[b]
            slot_in_bin[b] += 1
            order[p] = k * NOWN + nd
            local_pos_of[k * NOWN + nd] = p
        orders.append(order)

    table_row = np.empty(N, dtype=np.int64)
    for k in range(NC):
        gids = np.arange(k * NOWN, (k + 1) * NOWN)
        table_row[gids] = k * NPAD + local_pos_of[gids]

    for k in range(NC):
        es, ed, local_dst, a_edge, assign = per_core[k]
        j_of_edge = local_pos_of[ed] % 128  # slot within group
        g_of_edge = local_pos_of[ed] // 128
        rowA = table_row[es]
        idxA = np.zeros((G, CA * 128), dtype=np.int64)
        idxB = np.zeros((G, CA * 128), dtype=np.int64)
        vidx = np.zeros((G, 2, CA * 128), dtype=np.int64)
        dloc = np.full((G, KC, 128), 200.0, dtype=np.float32)
        for g in range(G):
            m = g_of_edge == g
            for half in (0, 1):
                hm = m & (a_edge if half == 0 else ~a_edge)
                rows = rowA[hm] - half * HALF
                js = j_of_edge[hm]
                n = len(rows)
                assert n <= CA * 128, (k, g, half, n)
                tgt = idxA if half == 0 else idxB
                tgt[g, :n] = rows
                vidx[g, half, :n] = local_pos_of[ed[hm]]
                for c in range(CA):
                    lo, hi = c * 128, min((c + 1) * 128, n)
                    if lo < hi:
                        dloc[g, half * CA + c, : hi - lo] = js[lo:hi]
        idxA_w = np.concatenate([_wrap_idx(idxA[g]) for g in range(G)], axis=1)
        idxB_w = np.concatenate([_wrap_idx(idxB[g]) for g in range(G)], axis=1)
        vidx_w = np.concatenate(
            [_wrap_idx(vidx[g, h]) for g in range(G) for h in (0, 1)], axis=1
        )
        # dstloc SBUF layout [128, G*KC]: column g*KC + c, partition = slot
        dloc_t = dloc.transpose(2, 0, 1).reshape(128, G * KC).copy()

        # permuted inputs
        order = orders[k]
        valid = order >= 0
        xs = np.zeros((NPAD, x.shape[1]), dtype=np.float32)
        xs[valid] = np.asarray(x, dtype=np.float32)[order[valid]]
        pre.append(
            {
                "xT0": np.ascontiguousarray(xs.T),
                "idxA": idxA_w,
                "idxB": idxB_w,
                "vidx": vidx_w,
                "dstloc": dloc_t,
                "order": order,
            }
        )
    return pre


# ------------------------------------------------------- device program


def build_program(cfg, use_tile_trace=False):
    import concourse.bass as bass
    import concourse.bacc as bacc
    import concourse.tile as tile
    import concourse.mybir as mybir

    f32 = mybir.dt.float32
    i16 = mybir.dt.int16
    fmm = getattr(mybir.dt, F32MM)
    NC, G, NPAD, NTAB, HALF = cfg["NC"], cfg["G"], cfg["NPAD"], cfg["NTAB"], cfg["HALF"]
    CA, KC = cfg["CA"], cfg["KC"]
    layers = cfg["layers"]
    AF = mybir.ActivationFunctionType
    OP = mybir.AluOpType

    nc = bacc.Bacc(
        "TRN2",
        target_bir_lowering=False,
        debug=False,
        num_devices=NC,
        enable_asserts=False,
    )

    F_IN0 = layers[0][0]
    xT0 = nc.dram_tensor("xT0", [F_IN0, NPAD], fmm, kind="ExternalInput").ap()
    idxA_d = nc.dram_tensor("idxA", [128, G * CA * 8], i16, kind="ExternalInput").ap()
    idxB_d = nc.dram_tensor("idxB", [128, G * CA * 8], i16, kind="ExternalInput").ap()
    vidx_d = nc.dram_tensor("vidx", [128, G * 2 * CA * 8], i16, kind="ExternalInput").ap()
    dstloc_d = nc.dram_tensor("dstloc", [128, G * KC], f32, kind="ExternalInput").ap()
    iota_d = nc.dram_tensor("iota", [128, 128], f32, kind="ExternalInput").ap()
    ident_d = nc.dram_tensor("ident", [128, 128], f32, kind="ExternalInput").ap()

    Ws, asb, adb, bb = [], [], [], []
    for li, (fin, H, D, fout, tw, _) in enumerate(layers):
        Ws.append(nc.dram_tensor(f"W{li}", [fin, fout], fmm, kind="ExternalInput").ap())
        asb.append(nc.dram_tensor(f"asb{li}", [128, fout], f32, kind="ExternalInput").ap())
        adb.append(nc.dram_tensor(f"adb{li}", [128, fout], f32, kind="ExternalInput").ap())
        bb.append(nc.dram_tensor(f"bb{li}", [128, fout], f32, kind="ExternalInput").ap())

    shard, tab, vstats, xT_next = [], [], [], []
    for li, (fin, H, D, fout, tw, _) in enumerate(layers):
        shard.append(nc.dram_tensor(f"shard{li}", [NPAD, tw], f32).ap())
        tab_space = "Shared" if NC > 4 else "Local"
        tab.append(nc.dram_tensor(f"tab{li}", [NTAB, tw], f32, addr_space=tab_space).ap())
        vstats.append(nc.dram_tensor(f"vstats{li}", [NPAD, 64], f32).ap())
        if li < len(layers) - 1:
            xT_next.append(nc.dram_tensor(f"xT{li + 1}", [fout, NPAD], fmm).ap())
    out_ext = nc.dram_tensor("out", [NPAD, layers[-1][3]], f32, kind="ExternalOutput").ap()

    with tile.TileContext(nc, trace_sim=use_tile_trace) as tc:
        with tc.tile_pool(name="const", bufs=1) as cpool:
            iota_sb = cpool.tile([128, 128], f32, tag="iota")
            ident_sb = cpool.tile([128, 128], f32, tag="ident")
            dstloc_sb = cpool.tile([128, G * KC], f32, tag="dstloc")
            idxA_sb = cpool.tile([128, G * CA * 8], i16, tag="idxA")
            idxB_sb = cpool.tile([128, G * CA * 8], i16, tag="idxB")
            vidx_sb = cpool.tile([128, G * 2 * CA * 8], i16, tag="vidx")
            nc.sync.dma_start(out=iota_sb[:, :], in_=iota_d)
            nc.sync.dma_start(out=ident_sb[:, :], in_=ident_d)
            nc.sync.dma_start(out=dstloc_sb[:, :], in_=dstloc_d)
            nc.sync.dma_start(out=idxA_sb[:, :], in_=idxA_d)
            nc.sync.dma_start(out=idxB_sb[:, :], in_=idxB_d)
            nc.sync.dma_start(out=vidx_sb[:, :], in_=vidx_d)

            for li, (fin, H, D, fout, tw, do_elu) in enumerate(layers):
                xT_cur = xT0 if li == 0 else xT_next[li - 1]
                FW = min(512, fout)
                NFC = fout // FW  # fout chunks in dense matmul
                NKC = fin // 128  # contraction chunks

                # ----------------- dense phase -----------------
                with (
                    tc.tile_pool(name=f"dW{li}", bufs=1) as wpool,
                    tc.tile_pool(name=f"dX{li}", bufs=3) as xpool,
                    tc.tile_pool(name=f"dH{li}", bufs=2) as hpool,
                    tc.tile_pool(name=f"dPS{li}", bufs=2, space="PSUM") as pspool,
                ):
                    wk = []
                    for kc in range(NKC):
                        w = wpool.tile([128, fout], fmm, tag=f"w{kc}")
                        nc.sync.dma_start(out=w[:, :], in_=Ws[li][kc * 128 : (kc + 1) * 128, :])
                        wk.append(w)
                    asrc_sb = wpool.tile([128, fout], f32, tag="asrc")
                    adst_sb = wpool.tile([128, fout], f32, tag="adst")
                    nc.sync.dma_start(out=asrc_sb[:, :], in_=asb[li])
                    nc.sync.dma_start(out=adst_sb[:, :], in_=adb[li])

                    for t in range(G):
                        lts = []
                        for kc in range(NKC):
                            lt = xpool.tile([128, 128], fmm, tag=f"lt{kc}")
                            nc.sync.dma_start(
                                out=lt[:, :],
                                in_=xT_cur[kc * 128 : (kc + 1) * 128, t * 128 : (t + 1) * 128],
                            )
                            lts.append(lt)
                        h_sb = hpool.tile([128, fout], f32, tag="h")
                        for fc in range(NFC):
                            ph = pspool.tile([128, FW], f32, tag="ph")
                            for kc in range(NKC):
                                nc.tensor.matmul(
                                    ph[:, :],
                                    lhsT=lts[kc][:, :],
                                    rhs=wk[kc][:, fc * FW : (fc + 1) * FW],
                                    start=(kc == 0),
                                    stop=(kc == NKC - 1),
                                )
                            nc.vector.tensor_copy(h_sb[:, fc * FW : (fc + 1) * FW], ph[:, :])
                        # attention logit pieces
                        tmp = hpool.tile([128, fout], f32, tag="tmp")
                        al = hpool.tile([128, 2 * H], f32, tag="al")
                        nc.vector.tensor_tensor(tmp[:, :], h_sb[:, :], asrc_sb[:, :], OP.mult)
                        nc.vector.tensor_reduce(
                            al[:, 0:H],
                            tmp[:, :].rearrange("p (h d) -> p h d", h=H),
                            mybir.AxisListType.X,
                            OP.add,
                        )
                        nc.vector.tensor_tensor(tmp[:, :], h_sb[:, :], adst_sb[:, :], OP.mult)
                        nc.vector.tensor_reduce(
                            al[:, H : 2 * H],
                            tmp[:, :].rearrange("p (h d) -> p h d", h=H),
                            mybir.AxisListType.X,
                            OP.add,
                        )
                        u_t = hpool.tile([128, 64], f32, tag="ut")
                        v_t = hpool.tile([128, 64], f32, tag="vt")
                        nc.vector.memset(u_t[:, :], 0.0)
                        nc.vector.memset(v_t[:, :], 0.0)
                        nc.scalar.activation(u_t[:, 0:H], al[:, 0:H], AF.Exp)
                        nc.scalar.activation(u_t[:, H : 2 * H], al[:, 0:H], AF.Exp, scale=0.2)
                        nc.scalar.activation(v_t[:, 0:H], al[:, H : 2 * H], AF.Exp)
                        nc.scalar.activation(v_t[:, H : 2 * H], al[:, H : 2 * H], AF.Exp, scale=0.2)
                        rows = slice(t * 128, (t + 1) * 128)
                        nc.sync.dma_start(out=shard[li][rows, 0:fout], in_=h_sb[:, :])
                        nc.sync.dma_start(out=shard[li][rows, fout : fout + 64], in_=u_t[:, :])
                        nc.sync.dma_start(out=vstats[li][rows, :], in_=v_t[:, :])

                # ----------------- allgather -----------------
                nc.gpsimd.collective_compute(
                    "AllGather",
                    OP.bypass,
                    replica_groups=[list(range(NC))],
                    ins=[shard[li].opt()],
                    outs=[tab[li].opt()],
                )

                # ----------------- aggregation phase -----------------
                with (
                    tc.tile_pool(name=f"aB{li}", bufs=1) as bpool,
                    tc.tile_pool(name=f"aS{li}", bufs=2) as spool,
                    tc.tile_pool(name=f"aH{li}", bufs=2) as hppool,
                    tc.tile_pool(name=f"aE{li}", bufs=2) as epool,
                    tc.tile_pool(name=f"aPM{li}", bufs=2, space="PSUM") as pmpool,
                    tc.tile_pool(name=f"aPT{li}", bufs=2, space="PSUM") as ptpool,
                ):
                    bias_sb = bpool.tile([128, fout], f32, tag="bias")
                    nc.sync.dma_start(out=bias_sb[:, :], in_=bb[li])
                    for g in range(G):
                        pm = pmpool.tile([128, fout], f32, tag="pm")
                        ps = pmpool.tile([128, max(H, 8)], f32, tag="ps")
                        for half in (0, 1):
                            S = spool.tile([128, CA * tw], f32, tag="S")
                            S3 = S[:, :].rearrange("p (c w) -> p c w", c=CA)
                            idx_sb = idxA_sb if half == 0 else idxB_sb
                            tab_half = tab[li][half * HALF : (half + 1) * HALF, :]
                            nc.gpsimd.dma_gather(
                                out_ap=S3,
                                in_ap=tab_half,
                                idxs_ap=idx_sb[:, g * CA * 8 : (g + 1) * CA * 8],
                                num_idxs=CA * 128,
                                num_idxs_reg=CA * 128,
                                elem_size=tw,
                            )
                            vt = spool.tile([128, CA * 64], f32, tag="vt")
                            vt3 = vt[:, :].rearrange("p (c w) -> p c w", c=CA)
                            nc.gpsimd.dma_gather(
                                out_ap=vt3,
                                in_ap=vstats[li],
                                idxs_ap=vidx_sb[
                                    :, (g * 2 + half) * CA * 8 : (g * 2 + half + 1) * CA * 8
                                ],
                                num_idxs=CA * 128,
                                num_idxs_reg=CA * 128,
                                elem_size=64,
                            )
                            # per-edge softmax numerator q = max(u1*v1, u2*v2), [128, CA*H]
                            q1 = spool.tile([128, CA * H], f32, tag="q1")
                            q = spool.tile([128, CA * H], fmm, tag="q")
                            nc.vector.tensor_tensor(
                                q1[:, :].rearrange("p (c h) -> p c h", c=CA),
                                S3[:, :, fout : fout + H],
                                vt3[:, :, 0:H],
                                OP.mult,
                            )
                            nc.vector.tensor_tensor(
                                q[:, :].rearrange("p (c h) -> p c h", c=CA),
                                S3[:, :, fout + H : fout + 2 * H],
                                vt3[:, :, H : 2 * H],
                                OP.mult,
                            )
                            nc.vector.tensor_tensor(q[:, :], q[:, :], q1[:, :], OP.max)
                            # 0/1 edge->dst incidence for these CA chunks
                            ind = spool.tile([128, CA * 128], fmm, tag="ind")
                            dl = dstloc_sb[:, g * KC + half * CA : g * KC + half * CA + CA]
                            nc.vector.tensor_tensor(
                                ind[:, :].rearrange("p (c i) -> p c i", c=CA),
                                iota_sb[:, :].unsqueeze(1).broadcast_to([128, CA, 128]),
                                dl.unsqueeze(2).broadcast_to([128, CA, 128]),
                                OP.is_equal,
                            )
                            # weighted rows q*h
                            hp = hppool.tile([128, CA * fout], fmm, tag="hp")
                            nc.vector.tensor_tensor(
                                hp[:, :].rearrange("p (c h d) -> p c h d", c=CA, h=H),
                                S3[:, :, 0:fout].rearrange("p c (h d) -> p c h d", h=H),
                                q[:, :]
                                .rearrange("p (c h) -> p c h", c=CA)
                                .unsqueeze(3)
                                .broadcast_to([128, CA, H, D]),
                                OP.mult,
                            )
                            for c in range(CA):
                                cc = half * CA + c
                                lhs = ind[:, c * 128 : (c + 1) * 128]
                                for fc in range(NFC):
                                    nc.tensor.matmul(
                                        pm[:, fc * FW : (fc + 1) * FW],
                                        lhsT=lhs,
                                        rhs=hp[:, c * fout + fc * FW : c * fout + (fc + 1) * FW],
                                        start=(cc == 0),
                                        stop=(cc == KC - 1),
                                    )
                                nc.tensor.matmul(
                                    ps[:, 0:H],
                                    lhsT=lhs,
                                    rhs=q[:, c * H : (c + 1) * H],
                                    start=(cc == 0),
                                    stop=(cc == KC - 1),
                                )
                        # ---- epilogue for group g ----
                        rec = epool.tile([128, H], f32, tag="rec")
                        nc.vector.tensor_scalar_add(rec[:, :], ps[:, 0:H], 1e-16)
                        nc.vector.reciprocal(rec[:, :], rec[:, :])
                        t1 = epool.tile([128, fout], f32, tag="t1")
                        nc.vector.tensor_tensor(
                            t1[:, :].rearrange("p (h d) -> p h d", h=H),
                            pm[:, :].rearrange("p (h d) -> p h d", h=H),
                            rec[:, :].unsqueeze(2).broadcast_to([128, H, D]),
                            OP.mult,
                        )
                        nc.vector.tensor_tensor(t1[:, :], t1[:, :], bias_sb[:, :], OP.add)
                        if do_elu:
                            zm = epool.tile([128, fout], f32, tag="zm")
                            ez = epool.tile([128, fout], f32, tag="ez")
                            nc.vector.tensor_scalar_min(zm[:, :], t1[:, :], 0.0)
                            nc.scalar.activation(ez[:, :], zm[:, :], AF.Exp)
                            nc.vector.tensor_scalar_max(t1[:, :], t1[:, :], 0.0)
                            nc.vector.scalar_tensor_tensor(
                                t1[:, :], t1[:, :], -1.0, ez[:, :], OP.add, OP.add
                            )
                        if li < len(layers) - 1:
                            for fb in range(fout // 128):
                                pt = ptpool.tile([128, 128], f32, tag="pt")
                                nc.tensor.transpose(
                                    pt[:, :], t1[:, fb * 128 : (fb + 1) * 128], ident_sb[:, :]
                                )
                                tb = epool.tile([128, 128], fmm, tag="tb")
                                nc.vector.tensor_copy(tb[:, :], pt[:, :])
                                nc.sync.dma_start(
                                    out=xT_next[li][
                                        fb * 128 : (fb + 1) * 128, g * 128 : (g + 1) * 128
                                    ],
                                    in_=tb[:, :],
                                )
                        else:
                            nc.sync.dma_start(
                                out=out_ext[g * 128 : (g + 1) * 128, :], in_=t1[:, :]
                            )

    nc.compile()
    return nc


# ------------------------------------------------------- weights packing


def make_in_maps(pre, inputs, cfg):
    NC = cfg["NC"]
    layers = cfg["layers"]
    iota = np.tile(np.arange(128, dtype=np.float32), (128, 1))
    ident = np.eye(128, dtype=np.float32)
    names = [("W1", "as1", "ad1", "b1"), ("W2", "as2", "ad2", "b2"), ("W3", "as3", "ad3", "b3")]
    const = {"iota": iota, "ident": ident}
    for li, (fin, H, D, fout, tw, _) in enumerate(layers):
        wn, an, dn, bn = names[li]
        const[f"W{li}"] = np.ascontiguousarray(np.asarray(inputs[wn], dtype=np.float32))
        const[f"asb{li}"] = np.tile(
            np.asarray(inputs[an], dtype=np.float32).reshape(1, fout), (128, 1)
        )
        const[f"adb{li}"] = np.tile(
            np.asarray(inputs[dn], dtype=np.float32).reshape(1, fout), (128, 1)
        )
        const[f"bb{li}"] = np.tile(
            np.asarray(inputs[bn], dtype=np.float32).reshape(1, fout), (128, 1)
        )
    in_maps = []
    for k in range(NC):
        m = dict(const)
        m["xT0"] = pre[k]["xT0"]
        m["idxA"] = pre[k]["idxA"]
        m["idxB"] = pre[k]["idxB"]
        m["vidx"] = pre[k]["vidx"]
        m["dstloc"] = pre[k]["dstloc"]
        in_maps.append(m)
    return in_maps


def assemble(results, pre, cfg):
    N, NC, NPAD = cfg["N"], cfg["NC"], cfg["NPAD"]
    fout = cfg["layers"][-1][3]
    full = np.empty((N, fout), dtype=np.float32)
    for k in range(NC):
        order = pre[k]["order"]
        valid = order >= 0
        full[order[valid]] = results[k]["out"][valid]
    return full


# ------------------------------------------------------- entry point

_BUILT = {}


def kernel(x, edge_index, W1, as1, ad1, b1, W2, as2, ad2, b2, W3, as3, ad3, b3):
    from concourse.bass_utils import run_bass_kernel_spmd

    inputs = dict(
        x=x, edge_index=edge_index, W1=W1, as1=as1, ad1=ad1, b1=b1,
        W2=W2, as2=as2, ad2=ad2, b2=b2, W3=W3, as3=as3, ad3=ad3, b3=b3,
    )
    cfg = make_cfg()
    pre = preprocess(np.asarray(x), np.asarray(edge_index), cfg)
    key = (cfg["N"], cfg["NC"], cfg["CA"])
    if key not in _BUILT:
        _BUILT[key] = build_program(cfg)
    nc = _BUILT[key]
    in_maps = make_in_maps(pre, inputs, cfg)
    res = run_bass_kernel_spmd(nc, in_maps, core_ids=list(range(cfg["NC"])))
    return assemble(res.results, pre, cfg)
